# revision 7
# baseline (speedup 1.0000x reference)
"""Trainium2 Bass kernel for the scatter_memory GRU memory-update module.

Computation (torch GRUCell semantics, chunk order r, z, n):
    current = memory[node_ids]                       # [B, H] gather
    gi = messages @ W_ih.T + b_ih ; gh = current @ W_hh.T + b_hh
    r = sigmoid(gi_r + gh_r) ; z = sigmoid(gi_z + gh_z)
    n = tanh(gi_n + r * gh_n)
    updated = (1 - z) * n + z * current
    new_memory = memory.at[node_ids].set(updated)    # scatter

Distribution: the B updated rows are sharded contiguously across 8
NeuronCores.  The gather/scatter over the 500k-row table and the
feature-major transposes run on the host; each core runs the GRU math on
its own [H, B/8] shard (feature dim H=128 sits on the SBUF partition
axis, so the GRU biases become per-partition vectors that fuse into the
ScalarEngine activation ops for free).

Engine layout (from several HW NTFF profiling rounds; at steady state
PE, ACT and DVE are all ~100% busy and the pace is ACT's
3 x (1024+352)/1.2ns per 1024-column chunk):
  PE   6 gate matmuls (bf16) + an identity-matmul accumulating
       t = r*(gh_n + b_hn) into p_in's still-open PSUM group, deferred
       one chunk so PE never waits on the ACT->DVE chain; ~9 dummy
       matmuls on zeroed tiles run during the DMA ramp so the PE clock
       (1.2 GHz for the first ~3.4us of activity, 2.4 GHz after) is
       warm when the first real chunk arrives
  ACT  sigmoid(r), sigmoid(z) per chunk + the previous chunk's tanh
       straight from PSUM
  DVE  t = (p_hn + b_hn)*r (the one PSUM-touching op) and the whole
       blend out = n + z*(h-n) as three bf16 SBUF TensorTensor ops in
       2x perf mode
  Pool UNUSED on purpose: concurrent GpSimd TensorTensor traffic slows
       DVE's 2x-mode ops 2-3.5x (SBUF port contention)
DMA order on the Sync HWDGE queue = first-need order: r/z gate weights,
first xh chunk, remaining weights, biases, then the chunk stream
interleaved with output writebacks.  (A previous revision queued five
big xh chunks ahead of the weights, pushing the first matmul to 12.5us.)
"""

import os
import sys

import numpy as np

for _p in ("/opt/trn_rl_repo", "/root/.axon_site/_ro/trn_rl_repo"):
    if os.path.isdir(_p) and _p not in sys.path:
        sys.path.insert(0, _p)

import ml_dtypes
from contextlib import ExitStack

import concourse.bass as bass
import concourse.tile as tile
from concourse import mybir
from concourse.bass_utils import run_bass_kernel_spmd

BF16 = ml_dtypes.bfloat16
import json as _json

N_CORES = 8
H = 128
NTILE = 1024

# exposed for test harnesses
LAST_RESULT = None

_NC_CACHE = {}


def _split_sync_waits(bir: dict) -> dict:
    """Hoist extra per-instruction semaphore waits into standalone
    EventSemaphore instructions.

    The walrus build in this container encodes at most ONE sync wait per
    instruction ("Too many sync wait commands" otherwise); Tile attaches
    one wait per dependency.  An engine-level standalone wait immediately
    before the instruction is semantically identical (the engine stalls
    either way), so keep the last wait inline and hoist the rest.
    """
    n = 0
    for fn in bir.get("functions", []):
        for blk in fn.get("blocks", []):
            out = []
            for inst in blk.get("instructions", []):
                si = inst.get("sync_info") or {}
                ow = si.get("on_wait") or []
                if len(ow) > 1:
                    for w in ow[:-1]:
                        n += 1
                        out.append({
                            "debug": inst.get("debug", 0),
                            "engine": inst["engine"],
                            "ins": [],
                            "outs": [],
                            "name": f"hoistw_{n}_{inst['name']}",
                            "opcode": "EventSemaphore",
                            "sync_info": {"on_update": [], "on_wait": [w]},
                        })
                    si["on_wait"] = [ow[-1]]
                out.append(inst)
            blk["instructions"] = out
    return bir


def _patch_json(nc: bass.Bass) -> None:
    orig = nc.to_json_bytes

    def patched() -> bytes:
        return _json.dumps(_split_sync_waits(_json.loads(orig()))).encode()

    nc.to_json_bytes = patched


def _chunk_plan(bpc: int) -> list[tuple[int, int]]:
    """512 head chunk (compute starts as soon as the small first DMA
    lands), 1024 middles, small tail so the final serial chain is
    short."""
    tail = [512, 256]
    mid = bpc - sum(tail)
    sizes = []
    while mid % NTILE:
        sizes.append(512)
        mid -= 512
    sizes += [NTILE] * (mid // NTILE)
    sizes += tail
    out = []
    pos = 0
    for c in sizes:
        out.append((pos, c))
        pos += c
    assert pos == bpc, (pos, bpc)
    return out


def _build_nc(bpc: int) -> bass.Bass:
    """Bass program for one core: GRU over a [H, bpc] feature-major shard."""
    assert bpc % 256 == 0
    f32 = mybir.dt.float32
    bf16 = mybir.dt.bfloat16
    sig = mybir.ActivationFunctionType.Sigmoid
    tanh = mybir.ActivationFunctionType.Tanh
    add_op = mybir.AluOpType.add
    mult_op = mybir.AluOpType.mult

    nc = bass.Bass()
    # wT column blocks: r_x, r_h, z_x, z_h, n_x, n_h, I (identity closes
    # the p_in accumulation group on PE); split into two DRAM tensors so
    # the first matmul only waits on the 2-gate-column transfer
    wTa = nc.declare_dram_parameter("wTa", [H, 2 * H], bf16, isOutput=False)
    wTb = nc.declare_dram_parameter("wTb", [H, 5 * H], bf16, isOutput=False)
    # xh packs x and h per partition so ONE DMA per chunk brings both
    xh = nc.declare_dram_parameter("xh", [H, 2, bpc], bf16, isOutput=False)
    # bias columns: 0 = b_ih_r + b_hh_r, 1 = b_ih_z + b_hh_z, 2 = b_hh_n, 3 = b_ih_n
    biases = nc.declare_dram_parameter("biases", [H, 4], f32, isOutput=False)
    outT = nc.declare_dram_parameter("outT", [H, bpc], bf16, isOutput=True)

    chunks = _chunk_plan(bpc)
    PREFETCH = 4  # xh chunk DMAs in flight ahead of compute

    with ExitStack() as ctx:
        tc = ctx.enter_context(tile.TileContext(nc))
        singles = ctx.enter_context(tc.tile_pool(name="singles", bufs=1))
        io = ctx.enter_context(tc.tile_pool(name="io", bufs=PREFETCH + 2))
        zs = ctx.enter_context(tc.tile_pool(name="zs", bufs=4))
        mids = ctx.enter_context(tc.tile_pool(name="mids", bufs=3))
        wide = ctx.enter_context(tc.tile_pool(name="wide", bufs=3))
        # 4 fixed [128, 1024] fp32 tags, 2 banks each -> all 8 banks
        psum = ctx.enter_context(tc.tile_pool(name="psum", bufs=1, space="PSUM"))

        # dummy sigmoid on a freshly-memset tile fires the ~1.3us ACT table
        # load immediately (zero DMA dependencies), so it overlaps the DMA
        # ramp instead of stalling the first real sigmoid
        warm_in = singles.tile([H, 1], f32)
        nc.vector.memset(warm_in, 0.0)
        warm_sb = singles.tile([H, 1], f32)
        nc.scalar.activation(out=warm_sb, in_=warm_in,
                             func=sig, bias=0.0, scale=1.0)

        # DMA queue split: xh chunks + output stream own the Sync HWDGE
        # queue (xh0 at its head); the weights + biases ride the
        # otherwise-idle GpSimd SWDGE queue, whose transfers proceed
        # concurrently on the shared SDMA engines.  A 2-row dummy
        # transfer leads the Sync queue to absorb the cold-path latency
        # ahead of xh0.
        warm_dma = singles.tile([H, 2], bf16)
        nc.sync.dma_start(out=warm_dma, in_=wTa[:, 0:2])
        wa_sb = singles.tile([H, 2 * H], bf16)
        nc.gpsimd.dma_start(out=wa_sb, in_=wTa[:, :])

        def issue_xh(ci: int):
            c0, csz = chunks[ci]
            t = io.tile([H, 2 * csz], bf16, tag="xh")
            nc.sync.dma_start(out=t, in_=xh[:, :, c0 : c0 + csz])
            return t

        xh_tiles: dict[int, object] = {0: issue_xh(0)}
        wb_sb = singles.tile([H, 5 * H], bf16)
        nc.gpsimd.dma_start(out=wb_sb, in_=wTb[:, :])
        b_sb = singles.tile([H, 4], f32)
        nc.gpsimd.dma_start(out=b_sb, in_=biases[:, :])
        for ci in range(1, min(PREFETCH, len(chunks))):
            xh_tiles[ci] = issue_xh(ci)

        eye_sb = wb_sb[:, 4 * H : 5 * H]

        def wsl(wcol: int):
            if wcol < 2:
                return wa_sb[:, wcol * H : (wcol + 1) * H]
            return wb_sb[:, (wcol - 2) * H : (wcol - 1) * H]

        # PE clock warmup: ~4us of dummy matmuls (zeroed operands, no DMA
        # dependencies) run while the weight/xh DMAs are in flight, so
        # the HAM activity window reaches the 2.4 GHz p-state before the
        # first real matmul.  They write the p_r PSUM banks, which chunk
        # 0 then WAW-reuses.
        wz_sb = singles.tile([H, H], bf16)
        nc.vector.memset(wz_sb, 0.0)
        xz_sb = singles.tile([H, 512], bf16)
        nc.vector.memset(xz_sb, 0.0)
        p_warm = psum.tile([H, NTILE], f32, tag="p_r")
        for _ in range(9):
            nc.tensor.matmul(p_warm[:, 0:512], wz_sb, xz_sb,
                             start=True, stop=True)

        # one-chunk software pipeline: chunk c's identity-matmul (closing
        # p_in's open group with + I @ t) and everything downstream of it
        # (tanh, blend, writeback) are emitted during chunk c+1
        pending_imm = []
        pending_tail = []

        n_ch = len(chunks)
        for ci, (c0, csz) in enumerate(chunks):
            if ci + PREFETCH < n_ch:
                xh_tiles[ci + PREFETCH] = issue_xh(ci + PREFETCH)
            xh_t = xh_tiles.pop(ci)
            x_sb = xh_t[:, 0:csz]
            h_sb = xh_t[:, csz : 2 * csz]

            # fixed-size PSUM tags so the pool footprint is exactly 8
            # banks regardless of the chunk-size taper
            p_r_full = psum.tile([H, NTILE], f32, tag="p_r")
            p_z_full = psum.tile([H, NTILE], f32, tag="p_z")
            p_in_full = psum.tile([H, NTILE], f32, tag="p_in")
            p_hn_full = psum.tile([H, NTILE], f32, tag="p_hn")
            p_r, p_z = p_r_full[:, 0:csz], p_z_full[:, 0:csz]
            p_in, p_hn = p_in_full[:, 0:csz], p_hn_full[:, 0:csz]

            # previous chunk's identity-matmul lands FIRST: it must
            # precede this chunk's p_in matmul in PE order (that matmul
            # WAR-waits on the previous tanh, which needs the I-matmul —
            # emitting it later would deadlock the PE queue)
            if pending_imm:
                pending_imm.pop(0)()

            # PE: weight-outer gate matmuls.  Group order r, z, hn, in
            # matches when each PSUM tag is freed by its consumer
            # (sig_r early, sig_z next, t mid, tanh of c-1 late).
            qs = [(q0, min(512, csz - q0)) for q0 in range(0, csz, 512)]
            for wcol, ptile, rhs, start, stop in (
                (0, p_r, x_sb, True, False),
                (1, p_r, h_sb, False, True),
                (2, p_z, x_sb, True, False),
                (3, p_z, h_sb, False, True),
                (5, p_hn, h_sb, True, True),
                (4, p_in, x_sb, True, False),   # group stays open for I @ t
            ):
                w_sl = wsl(wcol)
                for q0, qsz in qs:
                    qd = slice(q0, q0 + qsz)
                    nc.tensor.matmul(ptile[:, qd], w_sl, rhs[:, qd],
                                     start=start, stop=stop)

            # ACT: this chunk's sigmoids
            r_t = mids.tile([H, csz], bf16, tag="r")
            nc.scalar.activation(out=r_t, in_=p_r, func=sig,
                                 bias=b_sb[:, 0:1], scale=1.0)
            z_t = zs.tile([H, csz], bf16, tag="z")
            nc.scalar.activation(out=z_t, in_=p_z, func=sig,
                                 bias=b_sb[:, 1:2], scale=1.0)

            # previous chunk's tanh + blend + writeback (inputs all ready:
            # its I-matmul went to PE at the top of this chunk)
            if pending_tail:
                pending_tail.pop(0)()

            # DVE: t = (p_hn + b_hn) * r
            t_t = mids.tile([H, csz], bf16, tag="t")
            nc.vector.scalar_tensor_tensor(
                out=t_t, in0=p_hn, scalar=b_sb[:, 2:3], in1=r_t,
                op0=add_op, op1=mult_op)

            def imm(_p=p_in, _t=t_t, _csz=csz):
                for q0 in range(0, _csz, 512):
                    qsz = min(512, _csz - q0)
                    qd = slice(q0, q0 + qsz)
                    nc.tensor.matmul(_p[:, qd], eye_sb, _t[:, qd],
                                     start=False, stop=True)
            pending_imm.append(imm)

            def tail(_p=p_in, _z=z_t, _h=h_sb, _c0=c0, _csz=csz):
                nn = zs.tile([H, _csz], bf16, tag="n")
                nc.scalar.activation(out=nn, in_=_p, func=tanh,
                                     bias=b_sb[:, 3:4], scale=1.0)
                w_ch = wide.tile([H, _csz], bf16, tag="w")
                y_ch = wide.tile([H, _csz], bf16, tag="y")
                o_ch = wide.tile([H, _csz], bf16, tag="o")
                nc.vector.tensor_sub(out=w_ch, in0=_h, in1=nn)
                nc.vector.tensor_mul(out=y_ch, in0=_z, in1=w_ch)
                nc.vector.tensor_add(out=o_ch, in0=y_ch, in1=nn)
                nc.sync.dma_start(out=outT[:, _c0 : _c0 + _csz], in_=o_ch)
            pending_tail.append(tail)

        # drain the one-deep pipeline
        while pending_imm:
            pending_imm.pop(0)()
        while pending_tail:
            pending_tail.pop(0)()

    _patch_json(nc)
    return nc


def _get_nc(bpc: int) -> bass.Bass:
    if bpc not in _NC_CACHE:
        _NC_CACHE[bpc] = _build_nc(bpc)
    return _NC_CACHE[bpc]


def kernel(node_ids, messages, memory, W_ih, W_hh, b_ih, b_hh):
    global LAST_RESULT
    node_ids = np.asarray(node_ids)
    messages = np.asarray(messages, dtype=np.float32)
    memory = np.asarray(memory, dtype=np.float32)
    W_ih = np.asarray(W_ih, dtype=np.float32)
    W_hh = np.asarray(W_hh, dtype=np.float32)
    b_ih = np.asarray(b_ih, dtype=np.float32)
    b_hh = np.asarray(b_hh, dtype=np.float32)

    B = node_ids.shape[0]
    per = -(-B // N_CORES)                       # rows per core (unpadded)
    bpc = -(-per // 256) * 256                   # padded to 256 multiple
    nc = _get_nc(bpc)

    current = memory[node_ids]                   # [B, H] host gather

    bias = np.empty((H, 4), dtype=np.float32)
    bias[:, 0] = b_ih[0:H] + b_hh[0:H]
    bias[:, 1] = b_ih[H : 2 * H] + b_hh[H : 2 * H]
    bias[:, 2] = b_hh[2 * H : 3 * H]
    bias[:, 3] = b_ih[2 * H : 3 * H]

    # weight-outer column order: r_x, r_h | z_x, z_h, n_x, n_h, I
    wTa = np.empty((H, 2 * H), dtype=BF16)
    wTa[:, 0:H] = W_ih[0:H].T
    wTa[:, H : 2 * H] = W_hh[0:H].T
    wTb = np.zeros((H, 5 * H), dtype=BF16)
    for g in (1, 2):
        wTb[:, (2 * g - 2) * H : (2 * g - 1) * H] = W_ih[g * H : (g + 1) * H].T
        wTb[:, (2 * g - 1) * H : (2 * g) * H] = W_hh[g * H : (g + 1) * H].T
    wTb[:, 4 * H : 5 * H] = np.eye(H, dtype=BF16)

    in_maps = []
    for c in range(N_CORES):
        lo = c * per
        hi = min(lo + per, B)
        xh = np.zeros((H, 2, bpc), dtype=BF16)
        if hi > lo:
            xh[:, 0, : hi - lo] = messages[lo:hi].T
            xh[:, 1, : hi - lo] = current[lo:hi].T
        in_maps.append({"wTa": wTa, "wTb": wTb, "xh": xh, "biases": bias})

    res = run_bass_kernel_spmd(nc, in_maps, list(range(N_CORES)))
    LAST_RESULT = res

    updated = np.empty((B, H), dtype=np.float32)
    for c in range(N_CORES):
        lo = c * per
        hi = min(lo + per, B)
        if hi > lo:
            updated[lo:hi] = res.results[c]["outT"][:, : hi - lo].T.astype(np.float32)

    new_memory = memory.copy()
    new_memory[node_ids] = updated
    return new_memory


# revision 8
# speedup vs baseline: 1.0029x; 1.0029x over previous
"""Trainium2 Bass kernel for the scatter_memory GRU memory-update module.

Computation (torch GRUCell semantics, chunk order r, z, n):
    current = memory[node_ids]                       # [B, H] gather
    gi = messages @ W_ih.T + b_ih ; gh = current @ W_hh.T + b_hh
    r = sigmoid(gi_r + gh_r) ; z = sigmoid(gi_z + gh_z)
    n = tanh(gi_n + r * gh_n)
    updated = (1 - z) * n + z * current
    new_memory = memory.at[node_ids].set(updated)    # scatter

Distribution: the B updated rows are sharded contiguously across 8
NeuronCores.  The gather/scatter over the 500k-row table and the
feature-major transposes run on the host; each core runs the GRU math on
its own [H, B/8] shard (feature dim H=128 sits on the SBUF partition
axis, so the GRU biases become per-partition vectors that fuse into the
ScalarEngine activation ops for free).

Engine layout (from several HW NTFF profiling rounds; at steady state
PE, ACT and DVE are all ~100% busy and the pace is ACT's
3 x (1024+352)/1.2ns per 1024-column chunk):
  PE   6 gate matmuls (bf16) + an identity-matmul accumulating
       t = r*(gh_n + b_hn) into p_in's still-open PSUM group, deferred
       one chunk so PE never waits on the ACT->DVE chain; ~9 dummy
       matmuls on zeroed tiles run during the DMA ramp so the PE clock
       (1.2 GHz for the first ~3.4us of activity, 2.4 GHz after) is
       warm when the first real chunk arrives
  ACT  sigmoid(r), sigmoid(z) per chunk + the previous chunk's tanh
       straight from PSUM
  DVE  t = (p_hn + b_hn)*r (the one PSUM-touching op) and the whole
       blend out = n + z*(h-n) as three bf16 SBUF TensorTensor ops in
       2x perf mode
  Pool UNUSED on purpose: concurrent GpSimd TensorTensor traffic slows
       DVE's 2x-mode ops 2-3.5x (SBUF port contention)
DMA order on the Sync HWDGE queue = first-need order: r/z gate weights,
first xh chunk, remaining weights, biases, then the chunk stream
interleaved with output writebacks.  (A previous revision queued five
big xh chunks ahead of the weights, pushing the first matmul to 12.5us.)
"""

import os
import sys

import numpy as np

for _p in ("/opt/trn_rl_repo", "/root/.axon_site/_ro/trn_rl_repo"):
    if os.path.isdir(_p) and _p not in sys.path:
        sys.path.insert(0, _p)

import ml_dtypes
from contextlib import ExitStack

import concourse.bass as bass
import concourse.tile as tile
from concourse import mybir
from concourse.bass_utils import run_bass_kernel_spmd

BF16 = ml_dtypes.bfloat16
import json as _json

N_CORES = 8
H = 128
NTILE = 1024

# exposed for test harnesses
LAST_RESULT = None

_NC_CACHE = {}


def _split_sync_waits(bir: dict) -> dict:
    """Hoist extra per-instruction semaphore waits into standalone
    EventSemaphore instructions.

    The walrus build in this container encodes at most ONE sync wait per
    instruction ("Too many sync wait commands" otherwise); Tile attaches
    one wait per dependency.  An engine-level standalone wait immediately
    before the instruction is semantically identical (the engine stalls
    either way), so keep the last wait inline and hoist the rest.
    """
    n = 0
    for fn in bir.get("functions", []):
        for blk in fn.get("blocks", []):
            out = []
            for inst in blk.get("instructions", []):
                si = inst.get("sync_info") or {}
                ow = si.get("on_wait") or []
                if len(ow) > 1:
                    for w in ow[:-1]:
                        n += 1
                        out.append({
                            "debug": inst.get("debug", 0),
                            "engine": inst["engine"],
                            "ins": [],
                            "outs": [],
                            "name": f"hoistw_{n}_{inst['name']}",
                            "opcode": "EventSemaphore",
                            "sync_info": {"on_update": [], "on_wait": [w]},
                        })
                    si["on_wait"] = [ow[-1]]
                out.append(inst)
            blk["instructions"] = out
    return bir


def _patch_json(nc: bass.Bass) -> None:
    orig = nc.to_json_bytes

    def patched() -> bytes:
        return _json.dumps(_split_sync_waits(_json.loads(orig()))).encode()

    nc.to_json_bytes = patched


def _chunk_plan(bpc: int) -> list[tuple[int, int]]:
    """512 head chunk (compute starts as soon as the small first DMA
    lands), 1024 middles, small tail so the final serial chain is
    short."""
    tail = [512, 256]
    mid = bpc - sum(tail)
    sizes = []
    while mid % NTILE:
        sizes.append(512)
        mid -= 512
    sizes += [NTILE] * (mid // NTILE)
    sizes += tail
    out = []
    pos = 0
    for c in sizes:
        out.append((pos, c))
        pos += c
    assert pos == bpc, (pos, bpc)
    return out


def _build_nc(bpc: int) -> bass.Bass:
    """Bass program for one core: GRU over a [H, bpc] feature-major shard."""
    assert bpc % 256 == 0
    f32 = mybir.dt.float32
    bf16 = mybir.dt.bfloat16
    sig = mybir.ActivationFunctionType.Sigmoid
    tanh = mybir.ActivationFunctionType.Tanh
    add_op = mybir.AluOpType.add
    mult_op = mybir.AluOpType.mult

    nc = bass.Bass()
    # wT column blocks: r_x, r_h, z_x, z_h, n_x, n_h, I (identity closes
    # the p_in accumulation group on PE); split into two DRAM tensors so
    # the first matmul only waits on the 2-gate-column transfer
    wTa = nc.declare_dram_parameter("wTa", [H, 2 * H], bf16, isOutput=False)
    wTb = nc.declare_dram_parameter("wTb", [H, 5 * H], bf16, isOutput=False)
    # xh packs x and h per partition so ONE DMA per chunk brings both
    xh = nc.declare_dram_parameter("xh", [H, 2, bpc], bf16, isOutput=False)
    # bias columns: 0 = b_ih_r + b_hh_r, 1 = b_ih_z + b_hh_z, 2 = b_hh_n, 3 = b_ih_n
    biases = nc.declare_dram_parameter("biases", [H, 4], f32, isOutput=False)
    outT = nc.declare_dram_parameter("outT", [H, bpc], bf16, isOutput=True)

    chunks = _chunk_plan(bpc)
    PREFETCH = 4  # xh chunk DMAs in flight ahead of compute

    with ExitStack() as ctx:
        tc = ctx.enter_context(tile.TileContext(nc))
        singles = ctx.enter_context(tc.tile_pool(name="singles", bufs=1))
        io = ctx.enter_context(tc.tile_pool(name="io", bufs=PREFETCH + 2))
        zs = ctx.enter_context(tc.tile_pool(name="zs", bufs=4))
        mids = ctx.enter_context(tc.tile_pool(name="mids", bufs=3))
        wide = ctx.enter_context(tc.tile_pool(name="wide", bufs=3))
        # 4 fixed [128, 1024] fp32 tags, 2 banks each -> all 8 banks
        psum = ctx.enter_context(tc.tile_pool(name="psum", bufs=1, space="PSUM"))

        # dummy sigmoid on a freshly-memset tile fires the ~1.3us ACT table
        # load immediately (zero DMA dependencies), so it overlaps the DMA
        # ramp instead of stalling the first real sigmoid
        warm_in = singles.tile([H, 1], f32)
        nc.vector.memset(warm_in, 0.0)
        warm_sb = singles.tile([H, 1], f32)
        nc.scalar.activation(out=warm_sb, in_=warm_in,
                             func=sig, bias=0.0, scale=1.0)

        # Sync HWDGE queue in first-need order: a 2-row dummy transfer to
        # absorb the cold-path latency, r/z-gate weights, the first xh
        # chunk, the remaining weights, biases.  (Routing the weights via
        # the GpSimd SWDGE queue instead measured 12us SLOWER end-to-end
        # — keep everything on the one HWDGE queue.)
        warm_dma = singles.tile([H, 2], bf16)
        nc.sync.dma_start(out=warm_dma, in_=wTa[:, 0:2])
        wa_sb = singles.tile([H, 2 * H], bf16)
        nc.sync.dma_start(out=wa_sb, in_=wTa[:, :])

        def issue_xh(ci: int):
            c0, csz = chunks[ci]
            t = io.tile([H, 2 * csz], bf16, tag="xh")
            nc.sync.dma_start(out=t, in_=xh[:, :, c0 : c0 + csz])
            return t

        xh_tiles: dict[int, object] = {0: issue_xh(0)}
        wb_sb = singles.tile([H, 5 * H], bf16)
        nc.sync.dma_start(out=wb_sb, in_=wTb[:, :])
        b_sb = singles.tile([H, 4], f32)
        nc.sync.dma_start(out=b_sb, in_=biases[:, :])
        for ci in range(1, min(PREFETCH, len(chunks))):
            xh_tiles[ci] = issue_xh(ci)

        eye_sb = wb_sb[:, 4 * H : 5 * H]

        def wsl(wcol: int):
            if wcol < 2:
                return wa_sb[:, wcol * H : (wcol + 1) * H]
            return wb_sb[:, (wcol - 2) * H : (wcol - 1) * H]

        # PE clock warmup: ~4us of dummy matmuls (zeroed operands, no DMA
        # dependencies) run while the weight/xh DMAs are in flight, so
        # the HAM activity window reaches the 2.4 GHz p-state before the
        # first real matmul.  They write the p_r PSUM banks, which chunk
        # 0 then WAW-reuses.
        wz_sb = singles.tile([H, H], bf16)
        nc.vector.memset(wz_sb, 0.0)
        xz_sb = singles.tile([H, 512], bf16)
        nc.vector.memset(xz_sb, 0.0)
        p_warm = psum.tile([H, NTILE], f32, tag="p_r")
        for _ in range(9):
            nc.tensor.matmul(p_warm[:, 0:512], wz_sb, xz_sb,
                             start=True, stop=True)

        # one-chunk software pipeline: chunk c's identity-matmul (closing
        # p_in's open group with + I @ t) and everything downstream of it
        # (tanh, blend, writeback) are emitted during chunk c+1
        pending_imm = []
        pending_tail = []

        n_ch = len(chunks)
        for ci, (c0, csz) in enumerate(chunks):
            if ci + PREFETCH < n_ch:
                xh_tiles[ci + PREFETCH] = issue_xh(ci + PREFETCH)
            xh_t = xh_tiles.pop(ci)
            x_sb = xh_t[:, 0:csz]
            h_sb = xh_t[:, csz : 2 * csz]

            # fixed-size PSUM tags so the pool footprint is exactly 8
            # banks regardless of the chunk-size taper
            p_r_full = psum.tile([H, NTILE], f32, tag="p_r")
            p_z_full = psum.tile([H, NTILE], f32, tag="p_z")
            p_in_full = psum.tile([H, NTILE], f32, tag="p_in")
            p_hn_full = psum.tile([H, NTILE], f32, tag="p_hn")
            p_r, p_z = p_r_full[:, 0:csz], p_z_full[:, 0:csz]
            p_in, p_hn = p_in_full[:, 0:csz], p_hn_full[:, 0:csz]

            # previous chunk's identity-matmul lands FIRST: it must
            # precede this chunk's p_in matmul in PE order (that matmul
            # WAR-waits on the previous tanh, which needs the I-matmul —
            # emitting it later would deadlock the PE queue)
            if pending_imm:
                pending_imm.pop(0)()

            # PE: weight-outer gate matmuls.  Group order r, z, hn, in
            # matches when each PSUM tag is freed by its consumer
            # (sig_r early, sig_z next, t mid, tanh of c-1 late).
            qs = [(q0, min(512, csz - q0)) for q0 in range(0, csz, 512)]
            for wcol, ptile, rhs, start, stop in (
                (0, p_r, x_sb, True, False),
                (1, p_r, h_sb, False, True),
                (2, p_z, x_sb, True, False),
                (3, p_z, h_sb, False, True),
                (5, p_hn, h_sb, True, True),
                (4, p_in, x_sb, True, False),   # group stays open for I @ t
            ):
                w_sl = wsl(wcol)
                for q0, qsz in qs:
                    qd = slice(q0, q0 + qsz)
                    nc.tensor.matmul(ptile[:, qd], w_sl, rhs[:, qd],
                                     start=start, stop=stop)

            # ACT: this chunk's sigmoids
            r_t = mids.tile([H, csz], bf16, tag="r")
            nc.scalar.activation(out=r_t, in_=p_r, func=sig,
                                 bias=b_sb[:, 0:1], scale=1.0)
            z_t = zs.tile([H, csz], bf16, tag="z")
            nc.scalar.activation(out=z_t, in_=p_z, func=sig,
                                 bias=b_sb[:, 1:2], scale=1.0)

            # previous chunk's tanh + blend + writeback (inputs all ready:
            # its I-matmul went to PE at the top of this chunk)
            if pending_tail:
                pending_tail.pop(0)()

            # DVE: t = (p_hn + b_hn) * r
            t_t = mids.tile([H, csz], bf16, tag="t")
            nc.vector.scalar_tensor_tensor(
                out=t_t, in0=p_hn, scalar=b_sb[:, 2:3], in1=r_t,
                op0=add_op, op1=mult_op)

            def imm(_p=p_in, _t=t_t, _csz=csz):
                for q0 in range(0, _csz, 512):
                    qsz = min(512, _csz - q0)
                    qd = slice(q0, q0 + qsz)
                    nc.tensor.matmul(_p[:, qd], eye_sb, _t[:, qd],
                                     start=False, stop=True)
            pending_imm.append(imm)

            def tail(_p=p_in, _z=z_t, _h=h_sb, _c0=c0, _csz=csz):
                nn = zs.tile([H, _csz], bf16, tag="n")
                nc.scalar.activation(out=nn, in_=_p, func=tanh,
                                     bias=b_sb[:, 3:4], scale=1.0)
                w_ch = wide.tile([H, _csz], bf16, tag="w")
                y_ch = wide.tile([H, _csz], bf16, tag="y")
                o_ch = wide.tile([H, _csz], bf16, tag="o")
                nc.vector.tensor_sub(out=w_ch, in0=_h, in1=nn)
                nc.vector.tensor_mul(out=y_ch, in0=_z, in1=w_ch)
                nc.vector.tensor_add(out=o_ch, in0=y_ch, in1=nn)
                nc.sync.dma_start(out=outT[:, _c0 : _c0 + _csz], in_=o_ch)
            pending_tail.append(tail)

        # drain the one-deep pipeline
        while pending_imm:
            pending_imm.pop(0)()
        while pending_tail:
            pending_tail.pop(0)()

    _patch_json(nc)
    return nc


def _get_nc(bpc: int) -> bass.Bass:
    if bpc not in _NC_CACHE:
        _NC_CACHE[bpc] = _build_nc(bpc)
    return _NC_CACHE[bpc]


def kernel(node_ids, messages, memory, W_ih, W_hh, b_ih, b_hh):
    global LAST_RESULT
    node_ids = np.asarray(node_ids)
    messages = np.asarray(messages, dtype=np.float32)
    memory = np.asarray(memory, dtype=np.float32)
    W_ih = np.asarray(W_ih, dtype=np.float32)
    W_hh = np.asarray(W_hh, dtype=np.float32)
    b_ih = np.asarray(b_ih, dtype=np.float32)
    b_hh = np.asarray(b_hh, dtype=np.float32)

    B = node_ids.shape[0]
    per = -(-B // N_CORES)                       # rows per core (unpadded)
    bpc = -(-per // 256) * 256                   # padded to 256 multiple
    nc = _get_nc(bpc)

    current = memory[node_ids]                   # [B, H] host gather

    bias = np.empty((H, 4), dtype=np.float32)
    bias[:, 0] = b_ih[0:H] + b_hh[0:H]
    bias[:, 1] = b_ih[H : 2 * H] + b_hh[H : 2 * H]
    bias[:, 2] = b_hh[2 * H : 3 * H]
    bias[:, 3] = b_ih[2 * H : 3 * H]

    # weight-outer column order: r_x, r_h | z_x, z_h, n_x, n_h, I
    wTa = np.empty((H, 2 * H), dtype=BF16)
    wTa[:, 0:H] = W_ih[0:H].T
    wTa[:, H : 2 * H] = W_hh[0:H].T
    wTb = np.zeros((H, 5 * H), dtype=BF16)
    for g in (1, 2):
        wTb[:, (2 * g - 2) * H : (2 * g - 1) * H] = W_ih[g * H : (g + 1) * H].T
        wTb[:, (2 * g - 1) * H : (2 * g) * H] = W_hh[g * H : (g + 1) * H].T
    wTb[:, 4 * H : 5 * H] = np.eye(H, dtype=BF16)

    in_maps = []
    for c in range(N_CORES):
        lo = c * per
        hi = min(lo + per, B)
        xh = np.zeros((H, 2, bpc), dtype=BF16)
        if hi > lo:
            xh[:, 0, : hi - lo] = messages[lo:hi].T
            xh[:, 1, : hi - lo] = current[lo:hi].T
        in_maps.append({"wTa": wTa, "wTb": wTb, "xh": xh, "biases": bias})

    res = run_bass_kernel_spmd(nc, in_maps, list(range(N_CORES)))
    LAST_RESULT = res

    updated = np.empty((B, H), dtype=np.float32)
    for c in range(N_CORES):
        lo = c * per
        hi = min(lo + per, B)
        if hi > lo:
            updated[lo:hi] = res.results[c]["outT"][:, : hi - lo].T.astype(np.float32)

    new_memory = memory.copy()
    new_memory[node_ids] = updated
    return new_memory


# revision 10
# speedup vs baseline: 1.1874x; 1.1840x over previous
"""Trainium2 Bass kernel for the scatter_memory GRU memory-update module.

Computation (torch GRUCell semantics, chunk order r, z, n):
    current = memory[node_ids]                       # [B, H] gather
    gi = messages @ W_ih.T + b_ih ; gh = current @ W_hh.T + b_hh
    r = sigmoid(gi_r + gh_r) ; z = sigmoid(gi_z + gh_z)
    n = tanh(gi_n + r * gh_n)
    updated = (1 - z) * n + z * current
    new_memory = memory.at[node_ids].set(updated)    # scatter

Distribution: the B updated rows are sharded contiguously across 8
NeuronCores.  The gather/scatter over the 500k-row table and the
feature-major transposes run on the host; each core runs the GRU math on
its own [H, B/8] shard (feature dim H=128 sits on the SBUF partition
axis, so the GRU biases become per-partition vectors that fuse into the
ScalarEngine activation ops for free).

Engine layout (from several HW NTFF profiling rounds; at steady state
PE, ACT and DVE are all ~100% busy and the pace is ACT's
3 x (1024+352)/1.2ns per 1024-column chunk):
  PE   r,z gates as fp8e4m3 DoubleRow matmuls (x,h k-tiles fused, one
       pass per 512 columns; weights pre-scaled by 32 so they sit in
       e4m3's normal range, descaled for free by the sigmoid's scale
       operand), n-gate matmuls in bf16, + an identity-matmul accumulating
       t = r*(gh_n + b_hn) into p_in's still-open PSUM group, deferred
       one chunk so PE never waits on the ACT->DVE chain; ~9 dummy
       matmuls on zeroed tiles run during the DMA ramp so the PE clock
       (1.2 GHz for the first ~3.4us of activity, 2.4 GHz after) is
       warm when the first real chunk arrives
  ACT  sigmoid(r), sigmoid(z) per chunk + the previous chunk's tanh
       straight from PSUM
  DVE  t = (p_hn + b_hn)*r (the one PSUM-touching op) and the whole
       blend out = n + z*(h-n) as three bf16 SBUF TensorTensor ops in
       2x perf mode
  Pool UNUSED on purpose: concurrent GpSimd TensorTensor traffic slows
       DVE's 2x-mode ops 2-3.5x (SBUF port contention)
DMA order on the Sync HWDGE queue = first-need order: r/z gate weights,
first xh chunk, remaining weights, biases, then the chunk stream
interleaved with output writebacks.  (A previous revision queued five
big xh chunks ahead of the weights, pushing the first matmul to 12.5us.)
"""

import os
import sys

import numpy as np

for _p in ("/opt/trn_rl_repo", "/root/.axon_site/_ro/trn_rl_repo"):
    if os.path.isdir(_p) and _p not in sys.path:
        sys.path.insert(0, _p)

import ml_dtypes
from contextlib import ExitStack

import concourse.bass as bass
import concourse.tile as tile
from concourse import mybir
from concourse.bass_utils import run_bass_kernel_spmd

BF16 = ml_dtypes.bfloat16
import json as _json

N_CORES = 8
H = 128
NTILE = 1024

# exposed for test harnesses
LAST_RESULT = None

_NC_CACHE = {}


def _split_sync_waits(bir: dict) -> dict:
    """Hoist extra per-instruction semaphore waits into standalone
    EventSemaphore instructions.

    The walrus build in this container encodes at most ONE sync wait per
    instruction ("Too many sync wait commands" otherwise); Tile attaches
    one wait per dependency.  An engine-level standalone wait immediately
    before the instruction is semantically identical (the engine stalls
    either way), so keep the last wait inline and hoist the rest.
    """
    n = 0
    for fn in bir.get("functions", []):
        for blk in fn.get("blocks", []):
            out = []
            for inst in blk.get("instructions", []):
                si = inst.get("sync_info") or {}
                ow = si.get("on_wait") or []
                if len(ow) > 1:
                    for w in ow[:-1]:
                        n += 1
                        out.append({
                            "debug": inst.get("debug", 0),
                            "engine": inst["engine"],
                            "ins": [],
                            "outs": [],
                            "name": f"hoistw_{n}_{inst['name']}",
                            "opcode": "EventSemaphore",
                            "sync_info": {"on_update": [], "on_wait": [w]},
                        })
                    si["on_wait"] = [ow[-1]]
                out.append(inst)
            blk["instructions"] = out
    return bir


def _patch_json(nc: bass.Bass) -> None:
    orig = nc.to_json_bytes

    def patched() -> bytes:
        return _json.dumps(_split_sync_waits(_json.loads(orig()))).encode()

    nc.to_json_bytes = patched


def _chunk_plan(bpc: int) -> list[tuple[int, int]]:
    """512 head chunk (compute starts as soon as the small first DMA
    lands), 1024 middles, small tail so the final serial chain is
    short."""
    tail = [512, 256]
    mid = bpc - sum(tail)
    sizes = []
    while mid % NTILE:
        sizes.append(512)
        mid -= 512
    sizes += [NTILE] * (mid // NTILE)
    sizes += tail
    out = []
    pos = 0
    for c in sizes:
        out.append((pos, c))
        pos += c
    assert pos == bpc, (pos, bpc)
    return out


def _build_nc(bpc: int) -> bass.Bass:
    """Bass program for one core: GRU over a [H, bpc] feature-major shard."""
    assert bpc % 256 == 0
    f32 = mybir.dt.float32
    bf16 = mybir.dt.bfloat16
    sig = mybir.ActivationFunctionType.Sigmoid
    tanh = mybir.ActivationFunctionType.Tanh
    add_op = mybir.AluOpType.add
    mult_op = mybir.AluOpType.mult

    nc = bass.Bass()
    # wT column blocks: r_x, r_h, z_x, z_h, n_x, n_h, I (identity closes
    # the p_in accumulation group on PE); split into two DRAM tensors so
    # the first matmul only waits on the 2-gate-column transfer
    fp8 = mybir.dt.float8e4
    # DoubleRow lhsT for r and z: [k-tile (x|h), out-feature] pairs
    wdr = nc.declare_dram_parameter("wdr", [H, 4, H], fp8, isOutput=False)
    wTb = nc.declare_dram_parameter("wTb", [H, 3 * H], bf16, isOutput=False)
    # xh packs x and h per partition so ONE DMA per chunk brings both;
    # xh8 is the same data quantized to e4m3 for the DoubleRow gates
    xh = nc.declare_dram_parameter("xh", [H, 2, bpc], bf16, isOutput=False)
    xh8 = nc.declare_dram_parameter("xh8", [H, 2, bpc], fp8, isOutput=False)
    # bias columns: 0 = b_ih_r + b_hh_r, 1 = b_ih_z + b_hh_z, 2 = b_hh_n, 3 = b_ih_n
    biases = nc.declare_dram_parameter("biases", [H, 4], f32, isOutput=False)
    outT = nc.declare_dram_parameter("outT", [H, bpc], bf16, isOutput=True)

    chunks = _chunk_plan(bpc)
    PREFETCH = 4  # xh chunk DMAs in flight ahead of compute

    with ExitStack() as ctx:
        tc = ctx.enter_context(tile.TileContext(nc))
        singles = ctx.enter_context(tc.tile_pool(name="singles", bufs=1))
        io = ctx.enter_context(tc.tile_pool(name="io", bufs=PREFETCH + 2))
        zs = ctx.enter_context(tc.tile_pool(name="zs", bufs=4))
        mids = ctx.enter_context(tc.tile_pool(name="mids", bufs=3))
        wide = ctx.enter_context(tc.tile_pool(name="wide", bufs=3))
        # 4 fixed [128, 1024] fp32 tags, 2 banks each -> all 8 banks
        psum = ctx.enter_context(tc.tile_pool(name="psum", bufs=1, space="PSUM"))

        # dummy sigmoid on a freshly-memset tile fires the ~1.3us ACT table
        # load immediately (zero DMA dependencies), so it overlaps the DMA
        # ramp instead of stalling the first real sigmoid
        warm_in = singles.tile([H, 1], f32)
        nc.vector.memset(warm_in, 0.0)
        warm_sb = singles.tile([H, 1], f32)
        nc.scalar.activation(out=warm_sb, in_=warm_in,
                             func=sig, bias=0.0, scale=1.0)

        # Sync HWDGE queue in first-need order: a 2-row dummy transfer to
        # absorb the cold-path latency, r/z-gate weights, the first xh
        # chunk, the remaining weights, biases.  (Routing the weights via
        # the GpSimd SWDGE queue instead measured 12us SLOWER end-to-end
        # — keep everything on the one HWDGE queue.)
        warm_dma = singles.tile([H, 2], bf16)
        nc.sync.dma_start(out=warm_dma, in_=wTb[:, 0:2])
        wdr_sb = singles.tile([H, 4, H], fp8)
        nc.sync.dma_start(out=wdr_sb, in_=wdr[:, :, :])

        def issue_xh(ci: int):
            c0, csz = chunks[ci]
            t = io.tile([H, 2 * csz], bf16, tag="xh")
            nc.sync.dma_start(out=t, in_=xh[:, :, c0 : c0 + csz])
            t8 = io.tile([H, 2, csz], fp8, tag="xh8")
            nc.sync.dma_start(out=t8, in_=xh8[:, :, c0 : c0 + csz])
            return t, t8

        xh_tiles: dict[int, object] = {0: issue_xh(0)}
        wb_sb = singles.tile([H, 3 * H], bf16)
        nc.sync.dma_start(out=wb_sb, in_=wTb[:, :])
        b_sb = singles.tile([H, 4], f32)
        nc.sync.dma_start(out=b_sb, in_=biases[:, :])
        for ci in range(1, min(PREFETCH, len(chunks))):
            xh_tiles[ci] = issue_xh(ci)

        eye_sb = wb_sb[:, 2 * H : 3 * H]

        # PE clock warmup: ~4us of dummy matmuls (zeroed operands, no DMA
        # dependencies) run while the weight/xh DMAs are in flight, so
        # the HAM activity window reaches the 2.4 GHz p-state before the
        # first real matmul.  They write the p_r PSUM banks, which chunk
        # 0 then WAW-reuses.
        wz_sb = singles.tile([H, H], bf16)
        nc.vector.memset(wz_sb, 0.0)
        xz_sb = singles.tile([H, 512], bf16)
        nc.vector.memset(xz_sb, 0.0)
        p_warm = psum.tile([H, NTILE], f32, tag="p_r")
        for _ in range(9):
            nc.tensor.matmul(p_warm[:, 0:512], wz_sb, xz_sb,
                             start=True, stop=True)

        # one-chunk software pipeline: chunk c's identity-matmul (closing
        # p_in's open group with + I @ t) and everything downstream of it
        # (tanh, blend, writeback) are emitted during chunk c+1
        pending_imm = []
        pending_tail = []

        n_ch = len(chunks)
        for ci, (c0, csz) in enumerate(chunks):
            if ci + PREFETCH < n_ch:
                xh_tiles[ci + PREFETCH] = issue_xh(ci + PREFETCH)
            xh_t, xh8_t = xh_tiles.pop(ci)
            x_sb = xh_t[:, 0:csz]
            h_sb = xh_t[:, csz : 2 * csz]

            # fixed-size PSUM tags so the pool footprint is exactly 8
            # banks regardless of the chunk-size taper
            p_r_full = psum.tile([H, NTILE], f32, tag="p_r")
            p_z_full = psum.tile([H, NTILE], f32, tag="p_z")
            p_in_full = psum.tile([H, NTILE], f32, tag="p_in")
            p_hn_full = psum.tile([H, NTILE], f32, tag="p_hn")
            p_r, p_z = p_r_full[:, 0:csz], p_z_full[:, 0:csz]
            p_in, p_hn = p_in_full[:, 0:csz], p_hn_full[:, 0:csz]

            # previous chunk's identity-matmul lands FIRST: it must
            # precede this chunk's p_in matmul in PE order (that matmul
            # WAR-waits on the previous tanh, which needs the I-matmul —
            # emitting it later would deadlock the PE queue)
            if pending_imm:
                pending_imm.pop(0)()

            # PE: gate matmuls.  Group order r, z, hn, in matches when
            # each PSUM tag is freed by its consumer (sig_r early, sig_z
            # next, t mid, tanh of c-1 late).  r,z are single fp8
            # DoubleRow passes over both (x,h) k-tiles.
            qs = [(q0, min(512, csz - q0)) for q0 in range(0, csz, 512)]
            dr = mybir.MatmulPerfMode.DoubleRow
            for q0, qsz in qs:
                nc.tensor.matmul(p_r[:, q0 : q0 + qsz], wdr_sb[:, 0:2, :],
                                 xh8_t[:, :, q0 : q0 + qsz],
                                 start=True, stop=True, perf_mode=dr)
            for q0, qsz in qs:
                nc.tensor.matmul(p_z[:, q0 : q0 + qsz], wdr_sb[:, 2:4, :],
                                 xh8_t[:, :, q0 : q0 + qsz],
                                 start=True, stop=True, perf_mode=dr)
            for wcol, ptile, rhs, start, stop in (
                (1, p_hn, h_sb, True, True),
                (0, p_in, x_sb, True, False),   # group stays open for I @ t
            ):
                w_sl = wb_sb[:, wcol * H : (wcol + 1) * H]
                for q0, qsz in qs:
                    qd = slice(q0, q0 + qsz)
                    nc.tensor.matmul(ptile[:, qd], w_sl, rhs[:, qd],
                                     start=start, stop=stop)

            # ACT: this chunk's sigmoids
            r_t = mids.tile([H, csz], bf16, tag="r")
            nc.scalar.activation(out=r_t, in_=p_r, func=sig,
                                 bias=b_sb[:, 0:1], scale=1.0 / 32.0)
            z_t = zs.tile([H, csz], bf16, tag="z")
            nc.scalar.activation(out=z_t, in_=p_z, func=sig,
                                 bias=b_sb[:, 1:2], scale=1.0 / 32.0)

            # previous chunk's tanh + blend + writeback (inputs all ready:
            # its I-matmul went to PE at the top of this chunk)
            if pending_tail:
                pending_tail.pop(0)()

            # DVE: t = (p_hn + b_hn) * r
            t_t = mids.tile([H, csz], bf16, tag="t")
            nc.vector.scalar_tensor_tensor(
                out=t_t, in0=p_hn, scalar=b_sb[:, 2:3], in1=r_t,
                op0=add_op, op1=mult_op)

            def imm(_p=p_in, _t=t_t, _csz=csz):
                for q0 in range(0, _csz, 512):
                    qsz = min(512, _csz - q0)
                    qd = slice(q0, q0 + qsz)
                    nc.tensor.matmul(_p[:, qd], eye_sb, _t[:, qd],
                                     start=False, stop=True)
            pending_imm.append(imm)

            def tail(_p=p_in, _z=z_t, _h=h_sb, _c0=c0, _csz=csz):
                nn = zs.tile([H, _csz], bf16, tag="n")
                nc.scalar.activation(out=nn, in_=_p, func=tanh,
                                     bias=b_sb[:, 3:4], scale=1.0)
                w_ch = wide.tile([H, _csz], bf16, tag="w")
                y_ch = wide.tile([H, _csz], bf16, tag="y")
                o_ch = wide.tile([H, _csz], bf16, tag="o")
                nc.vector.tensor_sub(out=w_ch, in0=_h, in1=nn)
                nc.vector.tensor_mul(out=y_ch, in0=_z, in1=w_ch)
                nc.vector.tensor_add(out=o_ch, in0=y_ch, in1=nn)
                nc.sync.dma_start(out=outT[:, _c0 : _c0 + _csz], in_=o_ch)
            pending_tail.append(tail)

        # drain the one-deep pipeline
        while pending_imm:
            pending_imm.pop(0)()
        while pending_tail:
            pending_tail.pop(0)()

    _patch_json(nc)
    return nc


def _get_nc(bpc: int) -> bass.Bass:
    if bpc not in _NC_CACHE:
        _NC_CACHE[bpc] = _build_nc(bpc)
    return _NC_CACHE[bpc]


def kernel(node_ids, messages, memory, W_ih, W_hh, b_ih, b_hh):
    global LAST_RESULT
    node_ids = np.asarray(node_ids)
    messages = np.asarray(messages, dtype=np.float32)
    memory = np.asarray(memory, dtype=np.float32)
    W_ih = np.asarray(W_ih, dtype=np.float32)
    W_hh = np.asarray(W_hh, dtype=np.float32)
    b_ih = np.asarray(b_ih, dtype=np.float32)
    b_hh = np.asarray(b_hh, dtype=np.float32)

    B = node_ids.shape[0]
    per = -(-B // N_CORES)                       # rows per core (unpadded)
    bpc = -(-per // 256) * 256                   # padded to 256 multiple
    nc = _get_nc(bpc)

    current = memory[node_ids]                   # [B, H] host gather

    bias = np.empty((H, 4), dtype=np.float32)
    bias[:, 0] = b_ih[0:H] + b_hh[0:H]
    bias[:, 1] = b_ih[H : 2 * H] + b_hh[H : 2 * H]
    bias[:, 2] = b_hh[2 * H : 3 * H]
    bias[:, 3] = b_ih[2 * H : 3 * H]

    # r,z DoubleRow weights: 32x-scaled into e4m3's normal range (the
    # sigmoid descales via its scale operand); k-tile 0 = x-side,
    # k-tile 1 = h-side
    FP8 = ml_dtypes.float8_e4m3fn
    wdr = np.empty((H, 4, H), dtype=FP8)
    wdr[:, 0, :] = (32.0 * W_ih[0:H].T).astype(FP8)
    wdr[:, 1, :] = (32.0 * W_hh[0:H].T).astype(FP8)
    wdr[:, 2, :] = (32.0 * W_ih[H : 2 * H].T).astype(FP8)
    wdr[:, 3, :] = (32.0 * W_hh[H : 2 * H].T).astype(FP8)
    # n-gate weights (bf16) + the identity
    wTb = np.zeros((H, 3 * H), dtype=BF16)
    wTb[:, 0:H] = W_ih[2 * H : 3 * H].T
    wTb[:, H : 2 * H] = W_hh[2 * H : 3 * H].T
    wTb[:, 2 * H : 3 * H] = np.eye(H, dtype=BF16)

    in_maps = []
    for c in range(N_CORES):
        lo = c * per
        hi = min(lo + per, B)
        xh = np.zeros((H, 2, bpc), dtype=BF16)
        if hi > lo:
            xh[:, 0, : hi - lo] = messages[lo:hi].T
            xh[:, 1, : hi - lo] = current[lo:hi].T
        xh8 = xh.astype(FP8)
        in_maps.append({"wdr": wdr, "wTb": wTb, "xh": xh, "xh8": xh8,
                        "biases": bias})

    res = run_bass_kernel_spmd(nc, in_maps, list(range(N_CORES)))
    LAST_RESULT = res

    updated = np.empty((B, H), dtype=np.float32)
    for c in range(N_CORES):
        lo = c * per
        hi = min(lo + per, B)
        if hi > lo:
            updated[lo:hi] = res.results[c]["outT"][:, : hi - lo].T.astype(np.float32)

    new_memory = memory.copy()
    new_memory[node_ids] = updated
    return new_memory
